# revision 15
# baseline (speedup 1.0000x reference)
"""Trainium2 Bass kernel for an AttentionBlock (GroupNorm + MHA + proj + residual).

Shapes (hardcoded): x (16, 512, 32, 32) f32, 8 heads (ch=64), GN groups=32,
w_qkv (1536, 512), w_proj (512, 512).

Strategy: data-parallel over batch across 8 NeuronCores (2 batches/core, no
collectives). All matmuls run in fp8 (e4m3) with MatmulPerfMode.DoubleRow —
2 contraction rows per PE pass — which doubles PE throughput vs f32r/bf16.
Weights are pre-scaled by 32 on the host so they sit in e4m3's normal range;
the descale folds into the existing psum->sbuf bias-add ops. Scores use the
q/k channel dim split as 2x32 (DoubleRow pair over a 32-partition stationary),
attention@v keeps the ones-column denominator trick (out partitions = 65).
exp() writes fp8 directly from the ACT engine (scores max ~1.7, e^s << 240).
GroupNorm statistics use DVE reductions + a tiny block-diagonal f32r matmul,
rsqrt via DVE Newton iterations. The Pool engine (gpsimd) handles the x DMA
queue, residual bias prep, and the softmax-denominator partition broadcast.
"""
import numpy as np
import ml_dtypes
from contextlib import ExitStack

import concourse.bass as bass
import concourse.mybir as mybir
import concourse.tile as tile
from concourse import bacc
from concourse.bass_utils import run_bass_kernel_spmd

F32 = mybir.dt.float32
F32R = mybir.dt.float32r
FP8 = mybir.dt.float8e4
AF = mybir.ActivationFunctionType
OP = mybir.AluOpType
DR = mybir.MatmulPerfMode.DoubleRow

B, C, H, W = 16, 512, 32, 32
N = H * W            # 1024
NHEADS = 8
CH = C // NHEADS     # 64
NGROUPS = 32
GSIZE = C // NGROUPS  # 16 channels per group
EPS = 1e-5
NCORES = 8
BPC = B // NCORES    # batches per core = 2
NT = C // 128        # channel tiles per batch = 4
VW = NHEADS * (CH + 1)  # v cols incl ones = 520
VST = 576               # vv per-s-block stride: multiple of 32 (DoubleRow
                        # ldweights needs pair-stride % 32 == 0) and >= 551
                        # so head 7's 96-wide stationary slice stays in-block
SW = 32.0            # fp8 weight prescale
HS = 32.0            # hall (attn out) prescale
QS = float(CH) ** -0.25

_cached = {}


def _build(reps=1, num_devices=NCORES):
    nc = bacc.Bacc("TRN2", target_bir_lowering=False, debug=False,
                   num_devices=num_devices)

    xd = nc.dram_tensor("x", [BPC, C, N], F32, kind="ExternalInput").ap()
    wqk_d = nc.dram_tensor("wqk", [128, 4 * 1024], FP8, kind="ExternalInput").ap()
    wv_d = nc.dram_tensor("wv", [128, 4 * VW], FP8, kind="ExternalInput").ap()
    wp_d = nc.dram_tensor("wp", [128, 4 * C], FP8, kind="ExternalInput").ap()
    bqk_d = nc.dram_tensor("bqk", [128, 8], F32, kind="ExternalInput").ap()
    bv_d = nc.dram_tensor("bv_bc", [128, VW], F32, kind="ExternalInput").ap()
    bp_d = nc.dram_tensor("bp", [128, NT], F32, kind="ExternalInput").ap()
    gam_d = nc.dram_tensor("gamma_t", [128, NT], F32, kind="ExternalInput").ap()
    bet_d = nc.dram_tensor("beta_t", [128, NT], F32, kind="ExternalInput").ap()
    bd_d = nc.dram_tensor("blockdiag16", [128, 8], F32, kind="ExternalInput").ap()
    bc_d = nc.dram_tensor("bcast16", [8, 128], F32, kind="ExternalInput").ap()
    outd = nc.dram_tensor("out", [BPC, C, N], F32, kind="ExternalOutput").ap()

    with tile.TileContext(nc) as tc, ExitStack() as ctx:
        # ---- pools ----
        wpool = ctx.enter_context(tc.tile_pool(name="weights", bufs=1))
        xpool = ctx.enter_context(tc.tile_pool(name="x", bufs=2))
        xbpool = ctx.enter_context(tc.tile_pool(name="xb", bufs=2))
        xnpool = ctx.enter_context(tc.tile_pool(name="xn", bufs=2))
        qkpool = ctx.enter_context(tc.tile_pool(name="qk", bufs=2))
        vpool = ctx.enter_context(tc.tile_pool(name="v", bufs=2))
        hpool = ctx.enter_context(tc.tile_pool(name="h", bufs=2))
        ppool = ctx.enter_context(tc.tile_pool(name="p", bufs=2))
        opool = ctx.enter_context(tc.tile_pool(name="o", bufs=4))
        small = ctx.enter_context(tc.tile_pool(name="small", bufs=2))
        scr = ctx.enter_context(tc.tile_pool(name="scr", bufs=2))
        ps_sc = ctx.enter_context(tc.tile_pool(name="ps_sc", bufs=2, space="PSUM"))
        ps_h = ctx.enter_context(tc.tile_pool(name="ps_h", bufs=2, space="PSUM"))

        # ---- load weights + constants ----
        wqk_sb = wpool.tile([128, 4 * 1024], FP8, tag="wqk")
        nc.sync.dma_start(wqk_sb[:], wqk_d[:])
        wv_sb = wpool.tile([128, 4 * VW], FP8, tag="wv")
        nc.sync.dma_start(wv_sb[:], wv_d[:])
        wp_sb = wpool.tile([128, 4 * C], FP8, tag="wp")
        nc.sync.dma_start(wp_sb[:], wp_d[:])
        bqk = wpool.tile([128, 8], F32, tag="bqk")
        nc.sync.dma_start(bqk[:], bqk_d[:])
        bv = wpool.tile([128, VW], F32, tag="bv")
        nc.sync.dma_start(bv[:], bv_d[:])
        bp = wpool.tile([128, NT], F32, tag="bp")
        nc.sync.dma_start(bp[:], bp_d[:])
        gam = wpool.tile([128, NT], F32, tag="gam")
        nc.sync.dma_start(gam[:], gam_d[:])
        bet = wpool.tile([128, NT], F32, tag="bet")
        nc.sync.dma_start(bet[:], bet_d[:])
        bd16 = wpool.tile([128, 8], F32, tag="bd16")
        nc.sync.dma_start(bd16[:], bd_d[:])
        bc16 = wpool.tile([8, 128], F32, tag="bc16")
        nc.sync.dma_start(bc16[:], bc_d[:])

        wqk4 = wqk_sb[:].rearrange("p (j i m) -> p j i m", j=2, i=2)
        wv4 = wv_sb[:].rearrange("p (j i w) -> p j i w", j=2, i=2)
        wp4 = wp_sb[:].rearrange("p (j i m) -> p j i m", j=2, i=2)

        for b in [b for _ in range(reps) for b in range(BPC)]:
            # ---- load x (Pool DGE queue; SP queue is busy with stores) ----
            x_sb = xpool.tile([128, NT * N], F32, tag="x")
            for j in range(NT):
                nc.gpsimd.dma_start(x_sb[:, N * j:N * (j + 1)],
                                    xd[b, 128 * j:128 * (j + 1), :])

            # ---- GroupNorm stats ----
            stat = small.tile([128, 8], F32, tag="stat")
            sq = scr.tile([128, N], F32, tag="sq")
            for j in range(NT):
                nc.vector.reduce_sum(stat[:, j:j + 1], x_sb[:, N * j:N * (j + 1)],
                                     axis=mybir.AxisListType.X)
                nc.vector.scalar_tensor_tensor(
                    sq[:], x_sb[:, N * j:N * (j + 1)], 1.0,
                    x_sb[:, N * j:N * (j + 1)],
                    op0=OP.bypass, op1=OP.mult,
                    accum_out=stat[:, 4 + j:5 + j])
            ps_st = ps_sc.tile([8, 8], F32, tag="sc")
            nc.tensor.matmul(ps_st[:], bd16[:], stat[:], start=True, stop=True)
            inv = 1.0 / (GSIZE * N)
            mean8 = small.tile([8, 8], F32, tag="mean8")  # cols 0-3 mean, 4-7 rstd
            nc.vector.tensor_scalar_mul(mean8[:, 0:4], ps_st[:, 0:4], inv)
            ex2 = small.tile([8, 4], F32, tag="ex2")
            nc.vector.tensor_scalar_mul(ex2[:], ps_st[:, 4:8], inv)
            m2 = small.tile([8, 4], F32, tag="m2")
            nc.vector.tensor_mul(m2[:], mean8[:, 0:4], mean8[:, 0:4])
            veps = small.tile([8, 4], F32, tag="veps")
            nc.vector.scalar_tensor_tensor(veps[:], ex2[:], EPS, m2[:],
                                           op0=OP.add, op1=OP.subtract)
            r_cur = small.tile([8, 4], F32, tag="r0")
            nc.vector.tensor_scalar(r_cur[:], veps[:], -0.5, 1.5,
                                    op0=OP.mult, op1=OP.add)
            for it in range(3):
                t1 = small.tile([8, 4], F32, tag=f"nt1_{it}")
                nc.vector.tensor_mul(t1[:], r_cur[:], r_cur[:])
                t2 = small.tile([8, 4], F32, tag=f"nt2_{it}")
                nc.vector.scalar_tensor_tensor(t2[:], t1[:], -0.5, veps[:],
                                               op0=OP.mult, op1=OP.mult)
                t3 = small.tile([8, 4], F32, tag=f"nt3_{it}")
                nc.vector.tensor_scalar_add(t3[:], t2[:], 1.5)
                r_nxt = small.tile([8, 4], F32, tag=f"nr_{it}")
                nc.vector.tensor_mul(r_nxt[:], r_cur[:], t3[:])
                r_cur = r_nxt
            nc.vector.tensor_copy(mean8[:, 4:8], r_cur[:])
            ps_bc = ps_sc.tile([128, 8], F32, tag="sc")
            nc.tensor.matmul(ps_bc[:], bc16[:], mean8[:], start=True, stop=True)
            A_ch = small.tile([128, NT], F32, tag="A_ch")
            nc.vector.tensor_mul(A_ch[:], gam[:], ps_bc[:, 4:8])
            tB = small.tile([128, NT], F32, tag="tB")
            nc.vector.tensor_mul(tB[:], ps_bc[:, 0:4], A_ch[:])
            B_ch = small.tile([128, NT], F32, tag="B_ch")
            nc.vector.scalar_tensor_tensor(B_ch[:], tB[:], -1.0, bet[:],
                                           op0=OP.mult, op1=OP.add)
            # xn = A * x + B  (fp8 for the DoubleRow matmuls)
            xn = xnpool.tile([128, NT * N], FP8, tag="xn")
            for j in range(NT):
                nc.vector.tensor_scalar(xn[:, N * j:N * (j + 1)],
                                        x_sb[:, N * j:N * (j + 1)],
                                        A_ch[:, j:j + 1], B_ch[:, j:j + 1],
                                        op0=OP.mult, op1=OP.add)
            # residual + proj-bias, on Pool (sbuf only)
            xb = xbpool.tile([128, NT * N], F32, tag="xb")
            for j in range(NT):
                nc.gpsimd.tensor_scalar_add(xb[:, N * j:N * (j + 1)],
                                            x_sb[:, N * j:N * (j + 1)],
                                            bp[:, j:j + 1])

            xn4 = xn[:].rearrange("p (j i n) -> p j i n", j=2, i=2)

            # ---- q/k projection ----
            # qk_sb layout: q at col 2048*hb + 1024*i + n  (head h=4*hb+g on
            # partitions 32g..32g+31, ch 32i+r), k at 4096 + same.
            qk_sb = qkpool.tile([128, 8192], FP8, tag="qk")
            for o in range(8):
                pq = ps_h.tile([128, N], F32, tag="hacc", name=f"pq{o}")
                for kp in range(2):
                    for chk in range(4):
                        # psum zero regions are whole 2KB banks: start/stop on
                        # the first/last 256-col chunk touching each bank
                        nc.tensor.matmul(
                            pq[:, 256 * chk:256 * (chk + 1)],
                            wqk4[:, kp, :, 128 * o:128 * (o + 1)],
                            xn4[:, kp, :, 256 * chk:256 * (chk + 1)],
                            start=(kp == 0 and chk % 2 == 0),
                            stop=(kp == 1 and chk % 2 == 1), perf_mode=DR)
                nc.vector.tensor_scalar(qk_sb[:, N * o:N * (o + 1)],
                                        pq[:], QS / SW, bqk[:, o:o + 1],
                                        op0=OP.mult, op1=OP.add)

            # ---- v^T (+ones cols): vv[p, VST*j + vc] = v^T[s=128j+p, vc] ----
            # (31 pad cols per block so the 96-wide attnv stationary slice —
            # DoubleRow needs M % 32 == 0 — never crosses an s-block)
            vv = vpool.tile([128, 8 * VST], FP8, tag="vv")
            nc.gpsimd.memset(
                vv[:].rearrange("p (j w) -> p j w", j=8)[:, :, VW:VST], 0.0)
            for nb in range(8):
                pv = [ps_h.tile([128, 260], F32, tag="hacc", name=f"pv{nb}_{h}")
                      for h in range(2)]
                for kp in range(2):
                    for half in range(2):
                        for cq in range(2):
                            nc.tensor.matmul(
                                pv[half][:, 130 * cq:130 * (cq + 1)],
                                xn4[:, kp, :, 128 * nb:128 * (nb + 1)],
                                wv4[:, kp, :, 260 * half + 130 * cq:
                                    260 * half + 130 * (cq + 1)],
                                start=(kp == 0 and cq == 0),
                                stop=(kp == 1 and cq == 1), perf_mode=DR)
                for half in range(2):
                    nc.vector.scalar_tensor_tensor(
                        vv[:, VST * nb + 260 * half:VST * nb + 260 * (half + 1)],
                        pv[half][:], 1.0 / SW, bv[:, 260 * half:260 * (half + 1)],
                        op0=OP.mult, op1=OP.add)

            vv8 = vv[:].rearrange("p (j w) -> p j w", j=8)

            # ---- attention, head-sequential ----
            hall = hpool.tile([128, NT * N], FP8, tag="hall")
            for h in range(8):
                g, hb = h % 4, h // 4
                qh = qk_sb[32 * g:32 * (g + 1),
                           2048 * hb:2048 * (hb + 1)].rearrange(
                    "p (i n) -> p i n", i=2)
                kh = qk_sb[32 * g:32 * (g + 1),
                           4096 + 2048 * hb:4096 + 2048 * (hb + 1)].rearrange(
                    "p (i n) -> p i n", i=2)
                hps = ps_h.tile([96, N], F32, tag="hacc", name=f"hps{h}")
                p2 = None
                for j in range(8):
                    if j % 2 == 0:
                        p2 = ppool.tile([128, 2 * N], FP8, tag="p")
                    sc = ps_sc.tile([128, N], F32, tag="sc")
                    for chk in range(4):
                        nc.tensor.matmul(
                            sc[:, 256 * chk:256 * (chk + 1)],
                            kh[:, :, 128 * j:128 * (j + 1)],
                            qh[:, :, 256 * chk:256 * (chk + 1)],
                            start=(chk % 2 == 0), stop=(chk % 2 == 1),
                            perf_mode=DR, tile_position=(32 * g, 0))
                    nc.scalar.activation(p2[:, N * (j % 2):N * (j % 2 + 1)],
                                         sc[:], AF.Exp)
                    if j % 2 == 1:
                        j0 = j // 2
                        p2r = p2[:].rearrange("p (i n) -> p i n", i=2)
                        for chk in range(4):
                            nc.tensor.matmul(
                                hps[:, 256 * chk:256 * (chk + 1)],
                                vv8[:, 2 * j0:2 * j0 + 2, 65 * h:65 * h + 96],
                                p2r[:, :, 256 * chk:256 * (chk + 1)],
                                start=(j0 == 0 and chk % 2 == 0),
                                stop=(j0 == 3 and chk % 2 == 1), perf_mode=DR)
                # normalize: hall = (h * HS) / denom
                rec = small.tile([1, N], F32, tag="rec")
                nc.vector.reciprocal(rec[:], hps[64:65, :])
                rb = scr.tile([64, N], F32, tag="rb")
                nc.gpsimd.partition_broadcast(rb[:], rec[:])
                nc.vector.scalar_tensor_tensor(
                    hall[64 * (h % 2):64 * (h % 2) + 64,
                         N * (h // 2):N * (h // 2 + 1)],
                    hps[0:64, :], HS, rb[:], op0=OP.mult, op1=OP.mult)

            hall4 = hall[:].rearrange("p (j i n) -> p j i n", j=2, i=2)

            # ---- output projection + bias + residual ----
            for o in range(NT):
                pp = ps_h.tile([128, N], F32, tag="hacc", name=f"pp{o}")
                for kp in range(2):
                    for chk in range(4):
                        nc.tensor.matmul(
                            pp[:, 256 * chk:256 * (chk + 1)],
                            wp4[:, kp, :, 128 * o:128 * (o + 1)],
                            hall4[:, kp, :, 256 * chk:256 * (chk + 1)],
                            start=(kp == 0 and chk % 2 == 0),
                            stop=(kp == 1 and chk % 2 == 1), perf_mode=DR)
                for nh in range(2):
                    ot = opool.tile([128, 512], F32, tag="ot")
                    nc.vector.scalar_tensor_tensor(
                        ot[:], pp[:, 512 * nh:512 * (nh + 1)], 1.0 / (SW * HS),
                        xb[:, N * o + 512 * nh:N * o + 512 * (nh + 1)],
                        op0=OP.mult, op1=OP.add)
                    nc.sync.dma_start(
                        outd[b, 128 * o:128 * (o + 1), 512 * nh:512 * (nh + 1)],
                        ot[:])

    nc.compile()
    return nc


def _prep_shared(w_qkv, b_qkv, w_proj, b_proj, gamma, beta):
    f8 = ml_dtypes.float8_e4m3
    # qkv rows: head h -> q: 192h+c, k: 192h+64+c, v: 192h+128+c
    cols = np.empty(1024, np.int64)
    for oc in range(1024):
        o, po = divmod(oc, 128)
        g, r = divmod(po, 32)
        qk_off = 0 if o < 4 else 64
        hb, i = divmod(o % 4, 2)
        cols[oc] = 192 * (4 * hb + g) + qk_off + 32 * i + r
    wq = w_qkv[cols, :] * SW                      # [1024 oc, 512 c]
    wqk_arr = np.ascontiguousarray(
        wq.T.reshape(4, 128, 1024).transpose(1, 0, 2).reshape(128, 4096)
    ).astype(f8)
    bqk_arr = np.ascontiguousarray(
        (b_qkv[cols] * QS).reshape(8, 128).T).astype(np.float32)

    wv_full = np.zeros((VW, C), np.float32)
    bv_ext = np.zeros((VW,), np.float32)
    for vc in range(VW):
        hv, cc = divmod(vc, CH + 1)
        if cc < CH:
            wv_full[vc] = w_qkv[192 * hv + 128 + cc] * SW
            bv_ext[vc] = b_qkv[192 * hv + 128 + cc]
        else:
            bv_ext[vc] = 1.0
    wv_arr = np.ascontiguousarray(
        wv_full.T.reshape(4, 128, VW).transpose(1, 0, 2).reshape(128, 4 * VW)
    ).astype(f8)
    bv_bc = np.ascontiguousarray(
        np.broadcast_to(bv_ext, (128, VW))).astype(np.float32)

    wp_arr = np.ascontiguousarray(
        (w_proj * SW).T.reshape(4, 128, C).transpose(1, 0, 2).reshape(128, 4 * C)
    ).astype(f8)
    bp = np.ascontiguousarray(b_proj.reshape(NT, 128).T).astype(np.float32)
    gamma_t = np.ascontiguousarray(gamma.reshape(NT, 128).T).astype(np.float32)
    beta_t = np.ascontiguousarray(beta.reshape(NT, 128).T).astype(np.float32)
    blockdiag16 = np.kron(np.eye(8, dtype=np.float32),
                          np.ones((GSIZE, 1), np.float32))
    bcast16 = np.ascontiguousarray(blockdiag16.T)
    return dict(wqk=wqk_arr, bqk=bqk_arr, wv=wv_arr, bv_bc=bv_bc, wp=wp_arr,
                bp=bp, gamma_t=gamma_t, beta_t=beta_t,
                blockdiag16=blockdiag16, bcast16=bcast16)


def kernel(x, gamma, beta, w_qkv, b_qkv, w_proj, b_proj):
    x = np.asarray(x, dtype=np.float32)
    shared = _prep_shared(np.asarray(w_qkv, np.float32), np.asarray(b_qkv, np.float32),
                          np.asarray(w_proj, np.float32), np.asarray(b_proj, np.float32),
                          np.asarray(gamma, np.float32), np.asarray(beta, np.float32))
    x6 = x.reshape(B, C, N)
    in_maps = [dict(x=np.ascontiguousarray(x6[BPC * i:BPC * (i + 1)]), **shared)
               for i in range(NCORES)]
    if "nc" not in _cached:
        _cached["nc"] = _build()
    res = run_bass_kernel_spmd(_cached["nc"], in_maps, list(range(NCORES)))
    out = np.empty((B, C, N), np.float32)
    for i in range(NCORES):
        out[BPC * i:BPC * (i + 1)] = res.results[i]["out"]
    return out.reshape(B, C, H, W)


# revision 22
# speedup vs baseline: 1.0910x; 1.0910x over previous
"""Trainium2 Bass kernel for an AttentionBlock (GroupNorm + MHA + proj + residual).

Shapes (hardcoded): x (16, 512, 32, 32) f32, 8 heads (ch=64), GN groups=32,
w_qkv (1536, 512), w_proj (512, 512).

Strategy: data-parallel over batch across 8 NeuronCores (2 batches/core, no
collectives). All matmuls are fp8 (e4m3) MatmulPerfMode.DoubleRow — two
contraction rows per PE pass. Weights are pre-scaled by 32 on the host into
e4m3's normal range; descales fold into existing psum->sbuf bias ops. Scores
split the q/k channel dim as 2x32 (32-partition DoubleRow stationary); attn@v
keeps the ones-column softmax-denominator trick via a 96-wide stationary
(DoubleRow needs M%32==0; rows 65-95 are junk and ignored). exp() writes fp8
straight from ACT (softmax is exp-sum-normalize; scores max ~1.7 << ln 240).

Orchestration (the important part — the kernel is ACT(exp)-bound):
- attnv for pair p is emitted two j-steps after its exp, so the PE in-order
  stream never blocks on ACT; the last pair of head h flushes during head h+1.
- psum writes always alternate banks (chunk order 0,2,1,3); consecutive
  matmuls never target the same psum bank (same-bank back-to-back stalls the
  PE on accumulation-group drain).
- batch b+1's x-load / GroupNorm / qkv / v phases are interleaved into batch
  b's heads loop at head boundaries, so ACT only idles for the cold start.
- Pool (gpsimd) runs the x DMA queue, xn/xb elementwise prep, the vv pad
  memset and the denominator partition-broadcast; DVE keeps GN stats, psum
  evacuations and the normalize; ACT does nothing but exp.
"""
import numpy as np
import ml_dtypes
from contextlib import ExitStack

import concourse.bass as bass
import concourse.mybir as mybir
import concourse.tile as tile
from concourse import bacc
from concourse.bass_utils import run_bass_kernel_spmd

F32 = mybir.dt.float32
F32R = mybir.dt.float32r
FP8 = mybir.dt.float8e4
AF = mybir.ActivationFunctionType
OP = mybir.AluOpType
DR = mybir.MatmulPerfMode.DoubleRow

B, C, H, W = 16, 512, 32, 32
N = H * W            # 1024
NHEADS = 8
CH = C // NHEADS     # 64
NGROUPS = 32
GSIZE = C // NGROUPS  # 16
EPS = 1e-5
NCORES = 8
BPC = B // NCORES    # 2
NT = C // 128        # 4
VW = NHEADS * (CH + 1)  # 520
VST = 576            # vv s-block stride: %32==0 and >= 65*7+96
SW = 32.0            # fp8 weight prescale
HS = 32.0            # attn-out prescale
QS = float(CH) ** -0.25
CHK = (0, 2, 1, 3)   # 256-col chunk emission order: alternate psum banks

_cached = {}


def _build(reps=1, num_devices=NCORES):
    nc = bacc.Bacc("TRN2", target_bir_lowering=False, debug=False,
                   num_devices=num_devices)

    xd = nc.dram_tensor("x", [BPC, C, N], F32, kind="ExternalInput").ap()
    wqk_d = nc.dram_tensor("wqk", [128, 4 * 1024], FP8, kind="ExternalInput").ap()
    wv_d = nc.dram_tensor("wv", [128, 4 * VW], FP8, kind="ExternalInput").ap()
    wp_d = nc.dram_tensor("wp", [128, 4 * C], FP8, kind="ExternalInput").ap()
    bqk_d = nc.dram_tensor("bqk", [128, 8], F32, kind="ExternalInput").ap()
    bv_d = nc.dram_tensor("bv_bc", [128, VW], F32, kind="ExternalInput").ap()
    bp_d = nc.dram_tensor("bp", [128, NT], F32, kind="ExternalInput").ap()
    gam_d = nc.dram_tensor("gamma_t", [128, NT], F32, kind="ExternalInput").ap()
    bet_d = nc.dram_tensor("beta_t", [128, NT], F32, kind="ExternalInput").ap()
    bd_d = nc.dram_tensor("blockdiag16", [128, 8], F32, kind="ExternalInput").ap()
    bc_d = nc.dram_tensor("bcast16", [8, 128], F32, kind="ExternalInput").ap()
    outd = nc.dram_tensor("out", [BPC, C, N], F32, kind="ExternalOutput").ap()

    with tile.TileContext(nc) as tc, ExitStack() as ctx:
        wpool = ctx.enter_context(tc.tile_pool(name="weights", bufs=1))
        xpool = ctx.enter_context(tc.tile_pool(name="x", bufs=2))
        xbpool = ctx.enter_context(tc.tile_pool(name="xb", bufs=2))
        xnpool = ctx.enter_context(tc.tile_pool(name="xn", bufs=2))
        qkpool = ctx.enter_context(tc.tile_pool(name="qk", bufs=2))
        vpool = ctx.enter_context(tc.tile_pool(name="v", bufs=2))
        hpool = ctx.enter_context(tc.tile_pool(name="h", bufs=2))
        ppool = ctx.enter_context(tc.tile_pool(name="p", bufs=3))
        opool = ctx.enter_context(tc.tile_pool(name="o", bufs=4))
        small = ctx.enter_context(tc.tile_pool(name="small", bufs=2))
        scr = ctx.enter_context(tc.tile_pool(name="scr", bufs=2))
        ps_sc = ctx.enter_context(tc.tile_pool(name="ps_sc", bufs=2, space="PSUM"))
        ps_h = ctx.enter_context(tc.tile_pool(name="ps_h", bufs=2, space="PSUM"))

        wqk_sb = wpool.tile([128, 4 * 1024], FP8, tag="wqk")
        nc.sync.dma_start(wqk_sb[:], wqk_d[:])
        wv_sb = wpool.tile([128, 4 * VW], FP8, tag="wv")
        nc.sync.dma_start(wv_sb[:], wv_d[:])
        wp_sb = wpool.tile([128, 4 * C], FP8, tag="wp")
        nc.sync.dma_start(wp_sb[:], wp_d[:])
        bqk = wpool.tile([128, 8], F32, tag="bqk")
        nc.sync.dma_start(bqk[:], bqk_d[:])
        bv = wpool.tile([128, VW], F32, tag="bv")
        nc.sync.dma_start(bv[:], bv_d[:])
        bp = wpool.tile([128, NT], F32, tag="bp")
        nc.sync.dma_start(bp[:], bp_d[:])
        gam = wpool.tile([128, NT], F32, tag="gam")
        nc.sync.dma_start(gam[:], gam_d[:])
        bet = wpool.tile([128, NT], F32, tag="bet")
        nc.sync.dma_start(bet[:], bet_d[:])
        bd16 = wpool.tile([128, 8], F32, tag="bd16")
        nc.sync.dma_start(bd16[:], bd_d[:])
        bc16 = wpool.tile([8, 128], F32, tag="bc16")
        nc.sync.dma_start(bc16[:], bc_d[:])

        wqk4 = wqk_sb[:].rearrange("p (j i m) -> p j i m", j=2, i=2)
        wv4 = wv_sb[:].rearrange("p (j i w) -> p j i w", j=2, i=2)
        wp4 = wp_sb[:].rearrange("p (j i m) -> p j i m", j=2, i=2)

        class St:
            pass

        def emit_load(st):
            st.x_sb = xpool.tile([128, NT * N], F32, tag="x", name="x_sb")
            for j in range(NT):
                nc.gpsimd.dma_start(st.x_sb[:, N * j:N * (j + 1)],
                                    xd[st.b, 128 * j:128 * (j + 1), :])

        def emit_gn_stats(st):
            st.stat = small.tile([128, 8], F32, tag="stat", name="stat")
            sq = scr.tile([128, N], F32, tag="sq")
            for j in range(NT):
                nc.vector.reduce_sum(st.stat[:, j:j + 1],
                                     st.x_sb[:, N * j:N * (j + 1)],
                                     axis=mybir.AxisListType.X)
                nc.vector.scalar_tensor_tensor(
                    sq[:], st.x_sb[:, N * j:N * (j + 1)], 1.0,
                    st.x_sb[:, N * j:N * (j + 1)],
                    op0=OP.bypass, op1=OP.mult,
                    accum_out=st.stat[:, 4 + j:5 + j])

        def emit_gn(st):
            x_sb = st.x_sb
            stat = st.stat
            ps_st = ps_sc.tile([8, 8], F32, tag="sc")
            nc.tensor.matmul(ps_st[:], bd16[:], stat[:], start=True, stop=True)
            inv = 1.0 / (GSIZE * N)
            mean8 = small.tile([8, 8], F32, tag="mean8")
            nc.vector.tensor_scalar_mul(mean8[:, 0:4], ps_st[:, 0:4], inv)
            ex2 = small.tile([8, 4], F32, tag="ex2")
            nc.vector.tensor_scalar_mul(ex2[:], ps_st[:, 4:8], inv)
            m2 = small.tile([8, 4], F32, tag="m2")
            nc.vector.tensor_mul(m2[:], mean8[:, 0:4], mean8[:, 0:4])
            veps = small.tile([8, 4], F32, tag="veps")
            nc.vector.scalar_tensor_tensor(veps[:], ex2[:], EPS, m2[:],
                                           op0=OP.add, op1=OP.subtract)
            r_cur = small.tile([8, 4], F32, tag="r0")
            nc.vector.tensor_scalar(r_cur[:], veps[:], -0.5, 1.5,
                                    op0=OP.mult, op1=OP.add)
            for it in range(3):
                t1 = small.tile([8, 4], F32, tag=f"nt1_{it}")
                nc.vector.tensor_mul(t1[:], r_cur[:], r_cur[:])
                t2 = small.tile([8, 4], F32, tag=f"nt2_{it}")
                nc.vector.scalar_tensor_tensor(t2[:], t1[:], -0.5, veps[:],
                                               op0=OP.mult, op1=OP.mult)
                t3 = small.tile([8, 4], F32, tag=f"nt3_{it}")
                nc.vector.tensor_scalar_add(t3[:], t2[:], 1.5)
                r_nxt = small.tile([8, 4], F32, tag=f"nr_{it}")
                nc.vector.tensor_mul(r_nxt[:], r_cur[:], t3[:])
                r_cur = r_nxt
            nc.vector.tensor_copy(mean8[:, 4:8], r_cur[:])
            ps_bc = ps_sc.tile([128, 8], F32, tag="sc")
            nc.tensor.matmul(ps_bc[:], bc16[:], mean8[:], start=True, stop=True)
            A_ch = small.tile([128, NT], F32, tag="A_ch")
            nc.vector.tensor_mul(A_ch[:], gam[:], ps_bc[:, 4:8])
            tB = small.tile([128, NT], F32, tag="tB")
            nc.vector.tensor_mul(tB[:], ps_bc[:, 0:4], A_ch[:])
            B_ch = small.tile([128, NT], F32, tag="B_ch")
            nc.vector.scalar_tensor_tensor(B_ch[:], tB[:], -1.0, bet[:],
                                           op0=OP.mult, op1=OP.add)
            st.xn = xnpool.tile([128, NT * N], FP8, tag="xn", name="xn")
            st.xb = xbpool.tile([128, NT * N], F32, tag="xb", name="xb")
            for j in range(NT):
                nc.gpsimd.tensor_scalar(st.xn[:, N * j:N * (j + 1)],
                                        x_sb[:, N * j:N * (j + 1)],
                                        A_ch[:, j:j + 1], B_ch[:, j:j + 1],
                                        op0=OP.mult, op1=OP.add)
                nc.gpsimd.tensor_scalar_add(st.xb[:, N * j:N * (j + 1)],
                                            x_sb[:, N * j:N * (j + 1)],
                                            bp[:, j:j + 1])
            st.xn4 = st.xn[:].rearrange("p (j i n) -> p j i n", j=2, i=2)

        def emit_qk_alloc(st):
            st.qk_sb = qkpool.tile([128, 8192], FP8, tag="qk", name="qk_sb")

        def emit_qkv_o(st, o):
            pq = ps_sc.tile([128, N], F32, tag="sc", name=f"pq{o}")
            for kp in range(2):
                for chk in CHK:
                    nc.tensor.matmul(
                        pq[:, 256 * chk:256 * (chk + 1)],
                        wqk4[:, kp, :, 128 * o:128 * (o + 1)],
                        st.xn4[:, kp, :, 256 * chk:256 * (chk + 1)],
                        start=(kp == 0 and chk % 2 == 0),
                        stop=(kp == 1 and chk % 2 == 1), perf_mode=DR)
            nc.vector.tensor_scalar(st.qk_sb[:, N * o:N * (o + 1)],
                                    pq[:], QS / SW, bqk[:, o:o + 1],
                                    op0=OP.mult, op1=OP.add)

        def emit_v_alloc(st):
            st.vv = vpool.tile([128, 8 * VST], FP8, tag="vv", name="vv")
            nc.gpsimd.memset(
                st.vv[:].rearrange("p (j w) -> p j w", j=8)[:, :, VW:VST], 0.0)

        def emit_v_nb(st, nb):
            pv = [ps_sc.tile([128, 260], F32, tag="sc", name=f"pv{nb}_{h}")
                  for h in range(2)]
            for kp in range(2):
                for cq in range(2):
                    for half in range(2):  # alternate the two psum banks
                        nc.tensor.matmul(
                            pv[half][:, 130 * cq:130 * (cq + 1)],
                            st.xn4[:, kp, :, 128 * nb:128 * (nb + 1)],
                            wv4[:, kp, :, 260 * half + 130 * cq:
                                260 * half + 130 * (cq + 1)],
                            start=(kp == 0 and cq == 0),
                            stop=(kp == 1 and cq == 1), perf_mode=DR)
            for half in range(2):
                nc.vector.scalar_tensor_tensor(
                    st.vv[:, VST * nb + 260 * half:VST * nb + 260 * (half + 1)],
                    pv[half][:], 1.0 / SW, bv[:, 260 * half:260 * (half + 1)],
                    op0=OP.mult, op1=OP.add)

        def emit_heads(st, nxt):
            st.hall = hpool.tile([128, NT * N], FP8, tag="hall", name="hall")
            vvj = st.vv[:].rearrange("p (j w) -> p j w", j=8)
            pending = None  # (h, hps, p2 of pair3) awaiting attnv+normalize

            def attnv(h, hps, j0, p2):
                p2r = p2[:].rearrange("p (i n) -> p i n", i=2)
                for chk in CHK:
                    nc.tensor.matmul(
                        hps[:, 256 * chk:256 * (chk + 1)],
                        vvj[:, 2 * j0:2 * j0 + 2, 65 * h:65 * h + 96],
                        p2r[:, :, 256 * chk:256 * (chk + 1)],
                        start=(j0 == 0 and chk % 2 == 0),
                        stop=(j0 == 3 and chk % 2 == 1), perf_mode=DR)

            def normalize(h, hps):
                rec = small.tile([1, N], F32, tag="rec")
                nc.vector.reciprocal(rec[:], hps[64:65, :])
                rb = scr.tile([64, N], F32, tag="rb")
                nc.gpsimd.partition_broadcast(rb[:], rec[:])
                nc.vector.scalar_tensor_tensor(
                    st.hall[64 * (h % 2):64 * (h % 2) + 64,
                            N * (h // 2):N * (h // 2 + 1)],
                    hps[0:64, :], HS, rb[:], op0=OP.mult, op1=OP.mult)

            for h in range(8):
                # slot in the next batch's front-end work at head boundaries;
                # the PE-blocking pieces (GN matmuls, qkv) sit late enough
                # that their DVE/Pool/DMA inputs are already done
                if nxt is not None:
                    if h == 0:
                        emit_load(nxt)
                    elif h == 1:
                        emit_gn_stats(nxt)
                    elif h == 2:
                        emit_gn(nxt)
                        emit_qk_alloc(nxt)
                    elif 3 <= h <= 6:
                        emit_qkv_o(nxt, 2 * (h - 3))
                        emit_qkv_o(nxt, 2 * (h - 3) + 1)
                    else:
                        emit_v_alloc(nxt)
                        for nb in range(8):
                            emit_v_nb(nxt, nb)

                g, hb = h % 4, h // 4
                qh = st.qk_sb[32 * g:32 * (g + 1),
                              2048 * hb:2048 * (hb + 1)].rearrange(
                    "p (i n) -> p i n", i=2)
                kh = st.qk_sb[32 * g:32 * (g + 1),
                              4096 + 2048 * hb:4096 + 2048 * (hb + 1)].rearrange(
                    "p (i n) -> p i n", i=2)
                hps = ps_h.tile([96, N], F32, tag="hacc", name=f"hps{h}")
                p2s = {}
                for j in range(8):
                    if j % 2 == 0:
                        p2s[j // 2] = ppool.tile([128, 2 * N], FP8, tag="p", name=f"p2_{h}_{j}")
                    sc = ps_sc.tile([128, N], F32, tag="sc")
                    for chk in CHK:
                        nc.tensor.matmul(
                            sc[:, 256 * chk:256 * (chk + 1)],
                            kh[:, :, 128 * j:128 * (j + 1)],
                            qh[:, :, 256 * chk:256 * (chk + 1)],
                            start=(chk % 2 == 0), stop=(chk % 2 == 1),
                            perf_mode=DR, tile_position=(32 * g, 0))
                    nc.scalar.activation(p2s[j // 2][:, N * (j % 2):N * (j % 2 + 1)],
                                         sc[:], AF.Exp)
                    if j == 1 and pending is not None:
                        ph, phps, pp2 = pending
                        attnv(ph, phps, 3, pp2)
                        normalize(ph, phps)
                        pending = None
                    if j >= 3 and j % 2 == 1 and j < 8:
                        # pair (j-3)//2 exp'ed two steps ago: PE never waits
                        attnv(h, hps, (j - 3) // 2, p2s[(j - 3) // 2])
                pending = (h, hps, p2s[3])
            # flush the last head
            ph, phps, pp2 = pending
            attnv(ph, phps, 3, pp2)
            normalize(ph, phps)

        def emit_proj(st):
            hall4 = st.hall[:].rearrange("p (j i n) -> p j i n", j=2, i=2)
            for o in range(NT):
                pp = ps_sc.tile([128, N], F32, tag="sc", name=f"pp{o}")
                for kp in range(2):
                    for chk in CHK:
                        nc.tensor.matmul(
                            pp[:, 256 * chk:256 * (chk + 1)],
                            wp4[:, kp, :, 128 * o:128 * (o + 1)],
                            hall4[:, kp, :, 256 * chk:256 * (chk + 1)],
                            start=(kp == 0 and chk % 2 == 0),
                            stop=(kp == 1 and chk % 2 == 1), perf_mode=DR)
                for nh in range(2):
                    ot = opool.tile([128, 512], F32, tag="ot")
                    nc.vector.scalar_tensor_tensor(
                        ot[:], pp[:, 512 * nh:512 * (nh + 1)], 1.0 / (SW * HS),
                        st.xb[:, N * o + 512 * nh:N * o + 512 * (nh + 1)],
                        op0=OP.mult, op1=OP.add)
                    nc.sync.dma_start(
                        outd[st.b, 128 * o:128 * (o + 1),
                             512 * nh:512 * (nh + 1)],
                        ot[:])

        bs = [b for _ in range(reps) for b in range(BPC)]
        # cold start: batch 0's front-end inline
        st = St()
        st.b = bs[0]
        emit_load(st)
        emit_gn_stats(st)
        emit_gn(st)
        emit_qk_alloc(st)
        for o in range(8):
            emit_qkv_o(st, o)
        emit_v_alloc(st)
        for nb in range(8):
            emit_v_nb(st, nb)
        for idx in range(len(bs)):
            if idx + 1 < len(bs):
                nxt = St()
                nxt.b = bs[idx + 1]
            else:
                nxt = None
            emit_heads(st, nxt)
            emit_proj(st)
            if nxt is not None:
                st = nxt

    nc.compile()
    return nc


def _prep_shared(w_qkv, b_qkv, w_proj, b_proj, gamma, beta):
    f8 = ml_dtypes.float8_e4m3
    cols = np.empty(1024, np.int64)
    for oc in range(1024):
        o, po = divmod(oc, 128)
        g, r = divmod(po, 32)
        qk_off = 0 if o < 4 else 64
        hb, i = divmod(o % 4, 2)
        cols[oc] = 192 * (4 * hb + g) + qk_off + 32 * i + r
    wq = w_qkv[cols, :] * SW
    wqk_arr = np.ascontiguousarray(
        wq.T.reshape(4, 128, 1024).transpose(1, 0, 2).reshape(128, 4096)
    ).astype(f8)
    bqk_arr = np.ascontiguousarray(
        (b_qkv[cols] * QS).reshape(8, 128).T).astype(np.float32)

    wv_full = np.zeros((VW, C), np.float32)
    bv_ext = np.zeros((VW,), np.float32)
    for vc in range(VW):
        hv, cc = divmod(vc, CH + 1)
        if cc < CH:
            wv_full[vc] = w_qkv[192 * hv + 128 + cc] * SW
            bv_ext[vc] = b_qkv[192 * hv + 128 + cc]
        else:
            bv_ext[vc] = 1.0
    wv_arr = np.ascontiguousarray(
        wv_full.T.reshape(4, 128, VW).transpose(1, 0, 2).reshape(128, 4 * VW)
    ).astype(f8)
    bv_bc = np.ascontiguousarray(
        np.broadcast_to(bv_ext, (128, VW))).astype(np.float32)

    wp_arr = np.ascontiguousarray(
        (w_proj * SW).T.reshape(4, 128, C).transpose(1, 0, 2).reshape(128, 4 * C)
    ).astype(f8)
    bp = np.ascontiguousarray(b_proj.reshape(NT, 128).T).astype(np.float32)
    gamma_t = np.ascontiguousarray(gamma.reshape(NT, 128).T).astype(np.float32)
    beta_t = np.ascontiguousarray(beta.reshape(NT, 128).T).astype(np.float32)
    blockdiag16 = np.kron(np.eye(8, dtype=np.float32),
                          np.ones((GSIZE, 1), np.float32))
    bcast16 = np.ascontiguousarray(blockdiag16.T)
    return dict(wqk=wqk_arr, bqk=bqk_arr, wv=wv_arr, bv_bc=bv_bc, wp=wp_arr,
                bp=bp, gamma_t=gamma_t, beta_t=beta_t,
                blockdiag16=blockdiag16, bcast16=bcast16)


def kernel(x, gamma, beta, w_qkv, b_qkv, w_proj, b_proj):
    x = np.asarray(x, dtype=np.float32)
    shared = _prep_shared(np.asarray(w_qkv, np.float32), np.asarray(b_qkv, np.float32),
                          np.asarray(w_proj, np.float32), np.asarray(b_proj, np.float32),
                          np.asarray(gamma, np.float32), np.asarray(beta, np.float32))
    x6 = x.reshape(B, C, N)
    in_maps = [dict(x=np.ascontiguousarray(x6[BPC * i:BPC * (i + 1)]), **shared)
               for i in range(NCORES)]
    if "nc" not in _cached:
        _cached["nc"] = _build()
    res = run_bass_kernel_spmd(_cached["nc"], in_maps, list(range(NCORES)))
    out = np.empty((B, C, N), np.float32)
    for i in range(NCORES):
        out[BPC * i:BPC * (i + 1)] = res.results[i]["out"]
    return out.reshape(B, C, H, W)
